# revision 23
# baseline (speedup 1.0000x reference)
"""Trainium2 Bass kernel for nn_HarMABase contrastive+affiliation loss (v5).

B=4096, D=512, N_CLASSES=64, 8 NeuronCores, data-parallel over batch rows.

Per core c (rows r = 512c..512c+512), all feature math in fp8 DoubleRow:
  - tt8 = full text.T, fp8, in 512-col WINDOWS rotated per core so the
    core's own 512-col window is window 0; host unrotates column sums.
  - input DMAs dispatch from FOUR engines in parallel (sync carries the
    big streams in priority order tt8 -> nat8-img -> nat8-txt); outputs
    dispatch from the idle gpsimd engine.
  - main loop: 4 col groups of 1024 (2 windows), psum [128,1024]x3 bufs;
    exp on ACT with per-core shift G (max of a 256-col sample + 6); the
    colacc add on DVE carries accum_out running row sums (A[t,g]); the
    host recovers per-tile row sums by differencing A along t.  colacc
    ([128,1024] bf16 column-sum partials) exports RAW per group; host
    collapses partitions.
  - class sums (host-shipped one-hot fp8 x natural features fp8) into a
    2-bank psum alive all kernel; img half interleaved 2-ops-per-tile
    into groups 2-3, txt half 2-ops-per-tile in group 3 + post-loop.
  - affil split per direction and pipelined: as soon as the img class
    half lands, means->transpose->t.T matmul->exp overlap the rest of
    the main loop tail; txt direction follows.  Means scaled by
    8/(temp2*sqrt(1/temp))/cnt, cast fp8 (x8 boost keeps fp8 normals,
    undone by exp scale=1/8); raw s.T/t.T export bf16; zs collapse
    (cnt-weighted) happens on host from exp(s.T).
Host combines per-row/per-class partials into the scalar loss in float64.
"""

import functools
import os
import sys

import numpy as np

for _p in ("/root/.axon_site", "/root/.axon_site/_ro/trn_rl_repo"):
    if os.path.isdir(_p) and _p not in sys.path:
        sys.path.insert(0, _p)
if not os.path.isdir("/root/.axon_site/_ro/trn_rl_repo") and os.path.isdir(
    "/opt/trn_rl_repo"
):
    if "/opt/trn_rl_repo" not in sys.path:
        sys.path.insert(0, "/opt/trn_rl_repo")

N_CORES = 8
B = 4096
D = 512
NCLS = 64
SHARD = B // N_CORES  # 512
RT = SHARD // 128  # 4 row tiles per core
NT = B // 128  # 32 row tiles in the full batch
NOP = NT // 2  # 16 row-tile PAIRS (DoubleRow)
NW = B // 512  # 8 column windows of 512
GCH = 1024  # columns per psum group (2 banks)
NG = B // GCH  # 4 groups
LAST_RESULTS = None


@functools.lru_cache(maxsize=4)
def _compiled(temp: float, temp2: float):
    import concourse.bass as bass  # noqa: F401
    import concourse.tile as tile
    import concourse.bass_isa as bass_isa
    from concourse import bacc, mybir
    from concourse.masks import make_identity

    f32 = mybir.dt.float32
    bf16 = mybir.dt.bfloat16
    fp8 = mybir.dt.float8e4
    Exp = mybir.ActivationFunctionType.Exp
    X = mybir.AxisListType.X
    ALU = mybir.AluOpType
    DR = mybir.MatmulPerfMode.DoubleRow

    # fp8 operands arrive host-prescaled by sqrt(1/temp); means get an
    # extra x8 boost into fp8's normal range, undone by exp scale=1/8.
    BOOST = 8.0
    rt_st = float(np.sqrt(1.0 / temp))
    mean_imm = BOOST / (temp2 * rt_st)

    nc = bacc.Bacc(
        "TRN2",
        target_bir_lowering=False,
        debug=False,
        num_devices=N_CORES,
    )

    aux = nc.dram_tensor("aux", [64, 2], f32, kind="ExternalInput")
    is8 = nc.dram_tensor("is8", [128, 2, 2, SHARD], fp8, kind="ExternalInput")
    tt8 = nc.dram_tensor("tt8", [128, NW, 2, 2, 512], fp8, kind="ExternalInput")
    oh8 = nc.dram_tensor("oh8", [128, NOP, 2, NCLS], fp8, kind="ExternalInput")
    nat8 = nc.dram_tensor("nat8", [128, 2, NOP, 2, D], fp8, kind="ExternalInput")
    out = nc.dram_tensor("out", [128, 8], f32, kind="ExternalOutput")
    outz = nc.dram_tensor("outz", [128, RT, NG], f32, kind="ExternalOutput")
    outc = nc.dram_tensor("outc", [128, NG, GCH], bf16, kind="ExternalOutput")
    out3 = nc.dram_tensor("out3", [64, 1024], bf16, kind="ExternalOutput")
    out4 = nc.dram_tensor("out4", [64, 512], bf16, kind="ExternalOutput")
    outp = nc.dram_tensor("outp", [1, 512], f32, kind="ExternalOutput")

    with tile.TileContext(nc) as tc:
        with (
            tc.tile_pool(name="const", bufs=1) as const,
            tc.tile_pool(name="junk", bufs=3) as junkp,
            tc.tile_pool(name="colac", bufs=2) as colaccp,
            tc.tile_pool(name="stats", bufs=1) as statp,
        ):
            # ---------- input loads: parallel dispatch, then priority ----
            tt8_sb = const.tile([128, NW, 2, 2, 512], fp8, tag="tt8")
            is8_sb = const.tile([128, 2, 2, SHARD], fp8, tag="is8")
            aux_sb = const.tile([64, 2], f32, tag="aux")
            oh8_sb = const.tile([128, NOP, 2, NCLS], fp8, tag="oh8")
            nat8_sb = const.tile([128, 2, NOP, 2, D], fp8, tag="nat8")
            # phase A: first windows + small tensors, uncontended; later
            # phases gated by sync-engine drains so the DMA queues'
            # round-robin fair-sharing can't starve the critical path.
            nc.sync.dma_start(tt8_sb[:, 0:2], tt8[:, 0:2])
            nc.gpsimd.dma_start(is8_sb[:], is8[:, :, :, :])
            nc.gpsimd.dma_start(aux_sb[:], aux[:, :])
            nc.scalar.dma_start(oh8_sb[:], oh8[:, :, :, :])
            nc.sync.drain()
            nc.sync.dma_start(tt8_sb[:, 2:4], tt8[:, 2:4])
            nc.sync.dma_start(tt8_sb[:, 4:6], tt8[:, 4:6])
            nc.sync.dma_start(tt8_sb[:, 6:8], tt8[:, 6:8])
            nc.sync.drain()
            nc.sync.dma_start(nat8_sb[:, 0], nat8[:, 0])
            nc.sync.dma_start(nat8_sb[:, 1], nat8[:, 1])

            # ---------- constants / small setup ----------
            stage = const.tile([128, 8], f32, tag="stage")
            nc.vector.memset(stage[:], 0.0)
            ident = const.tile([64, 64], f32, tag="ident")
            make_identity(nc, ident[:])
            ones1 = const.tile([128, 1], bf16, tag="ones1")
            nc.vector.memset(ones1[:], 1.0)
            pdg_sb = statp.tile([1, 512], f32, tag="pdg_sb")
            # dummy ISA op: pulls the gpsimd library load into the
            # uncontended startup window (the real all-reduce otherwise
            # waits on a library-load DMA starved by input traffic).
            nc.gpsimd.partition_all_reduce(
                stage[:, 7:8],
                stage[:, 7:8],
                channels=128,
                reduce_op=bass_isa.ReduceOp.max,
            )

            Gt = statp.tile([128, 2], f32, tag="Gt")  # col0 G, col1 -G
            zbG = statp.tile([128, RT, NG], f32, tag="zbG")

            with tc.tile_pool(name="psAux", bufs=1, space="PSUM") as psAux:
                # 2 banks, alive all kernel: diag [1,512] + class sums
                # [64,1024] -> transposed means (after each half retires).
                pcls = psAux.tile([128, 1024], f32, tag="pcls", name="pcls")

                d4 = junkp.tile([128, 2, 2, SHARD], bf16, tag="d4", name="d4")

                # class-sum ops, spread across late main-loop row tiles:
                # img (co=0) during groups 2-3, txt (co=512) g3 + post.
                class_ops = [(0, op) for op in range(NOP)] + [
                    (512, op) for op in range(NOP)
                ]

                def emit_class(n):
                    while n > 0 and class_ops:
                        co, op = class_ops.pop(0)
                        src = nat8_sb[:, 0] if co == 0 else nat8_sb[:, 1]
                        nc.tensor.matmul(
                            pcls[0:64, co : co + 512],
                            oh8_sb[:, op, :, :],
                            src[:, op, :, :],
                            start=(op == 0),
                            stop=(op == NOP - 1),
                            perf_mode=DR,
                        )
                        n -= 1

                mFb = statp.tile([64, 1024], f32, tag="mFb")
                mT8 = statp.tile([128, 2, 2, 2, NCLS], fp8, tag="mT8")

                def emit_means(di):
                    # means for direction di (0=img,1=txt) + transpose +
                    # fp8 cast into mT8[:, di]; pm reuses the retired
                    # class psum half.
                    co = 512 * di
                    nc.vector.scalar_tensor_tensor(
                        out=mFb[:, co : co + 512],
                        in0=pcls[0:64, co : co + 512],
                        scalar=mean_imm,
                        in1=aux_sb[:, 0:1].broadcast_to([64, 512]),
                        op0=ALU.mult,
                        op1=ALU.mult,
                    )
                    pm = pcls[:, co : co + 256]
                    for c4 in range(4):
                        nc.tensor.transpose(
                            pm[:, 64 * c4 : 64 * (c4 + 1)],
                            mFb[:, co + 128 * c4 : co + 128 * (c4 + 1)],
                            ident[:, :],
                        )
                    nc.vector.tensor_copy(
                        mT8[:, di].rearrange("p b c d -> p (b c d)"), pm[:]
                    )

                # PE warmup during the DMA wait: junk matmuls ramp the
                # pstate so the first real matmuls run at full clock.
                for w in range(14):
                    nc.tensor.matmul(
                        pcls[0:64, 512 : 512 + 64],
                        ident[:, :],
                        ident[:, :],
                        start=True,
                        stop=True,
                    )

                with tc.tile_pool(name="psumB", bufs=3, space="PSUM") as psumB:
                    for g in range(NG):
                        colacc = colaccp.tile(
                            [128, GCH], bf16, tag="colacc", name="colacc"
                        )
                        for t in range(RT):
                            ps = psumB.tile([128, GCH], f32, tag="mm", name="ps")
                            for j in range(2):
                                for kp in range(2):
                                    nc.tensor.matmul(
                                        ps[:, 512 * j : 512 * (j + 1)],
                                        is8_sb[:, kp, :, 128 * t : 128 * (t + 1)],
                                        tt8_sb[:, 2 * g + j, kp, :, :],
                                        start=(kp == 0),
                                        stop=(kp == 1),
                                        perf_mode=DR,
                                    )
                                if g == 0 and t == 0 and j == 0:
                                    # G = max(256-col sample) + 6; chain
                                    # completes while window 1 multiplies.
                                    nc.vector.tensor_reduce(
                                        Gt[:, 0:1], ps[:, 0:256], axis=X, op=ALU.max
                                    )
                                    nc.vector.tensor_scalar_add(
                                        Gt[:, 0:1], Gt[:, 0:1], 6.0
                                    )
                                    nc.gpsimd.partition_all_reduce(
                                        Gt[:, 0:1],
                                        Gt[:, 0:1],
                                        channels=128,
                                        reduce_op=bass_isa.ReduceOp.max,
                                    )
                                    nc.vector.tensor_scalar_mul(
                                        Gt[:, 1:2], Gt[:, 0:1], -1.0
                                    )
                                    nc.vector.tensor_copy(stage[:, 4:5], Gt[:, 0:1])
                                    # diag products on the idle gpsimd
                                    nc.gpsimd.tensor_tensor(
                                        d4[:], is8_sb[:], tt8_sb[:, 0], op=ALU.mult
                                    )
                            if g == 1 and t in (2, 3):
                                # diag ones-matmuls; must finish before the
                                # class img half resets pcls[:, 0:512]
                                for k in (0, 1) if t == 2 else (2, 3):
                                    nc.tensor.matmul(
                                        pcls[0:1, 0:512],
                                        ones1[:, 0:1],
                                        d4[:, k // 2, k % 2, :],
                                        start=(k == 0),
                                        stop=(k == 3),
                                    )
                                if t == 3:
                                    nc.vector.tensor_copy(
                                        pdg_sb[:], pcls[0:1, 0:512]
                                    )
                                    nc.gpsimd.dma_start(outp[:, :], pdg_sb[:])
                            jk = junkp.tile([128, GCH], bf16, tag="jexp", name="jexp")
                            # rowsums: t==1 via ACT accumulator, others via
                            # DVE accumulators on the colacc op (running
                            # sums; host differences along t).
                            if t == 1:
                                nc.scalar.activation(
                                    jk[:],
                                    ps[:],
                                    Exp,
                                    bias=Gt[:, 1:2],
                                    accum_out=zbG[:, t, g : g + 1],
                                )
                                nc.vector.tensor_tensor(
                                    colacc[:], colacc[:], jk[:], op=ALU.add
                                )
                            elif t == 0:
                                nc.scalar.activation(jk[:], ps[:], Exp, bias=Gt[:, 1:2])
                                nc.vector.tensor_scalar(
                                    colacc[:],
                                    jk[:],
                                    1.0,
                                    0.0,
                                    op0=ALU.mult,
                                    op1=ALU.add,
                                    accum_out=zbG[:, t, g : g + 1],
                                )
                            else:
                                nc.scalar.activation(jk[:], ps[:], Exp, bias=Gt[:, 1:2])
                                nc.vector.scalar_tensor_tensor(
                                    out=colacc[:],
                                    in0=jk[:],
                                    scalar=1.0,
                                    in1=colacc[:],
                                    op0=ALU.mult,
                                    op1=ALU.add,
                                    accum_out=zbG[:, t, g : g + 1],
                                )
                        nc.gpsimd.dma_start(outc[:, g], colacc[:])
                    emit_class(16)
                    nc.gpsimd.dma_start(outz[:], zbG[:])

                # ---------- affil, per-direction pipeline ----------
                with tc.tile_pool(name="psA", bufs=1, space="PSUM") as psA:
                    pAff = psA.tile([64, 2 * SHARD], f32, tag="pAff", name="pAff")
                    o3b = statp.tile([64, 1024], bf16, tag="o3b")
                    Esc = junkp.tile([64, SHARD], bf16, tag="Esc", name="Esc")
                    jEt = junkp.tile([64, SHARD], bf16, tag="jEt", name="jEt")

                    # t-direction: needs img means (class img half done at
                    # end of g3 interleave)
                    emit_means(0)
                    emit_class(4)
                    for kp in range(2):
                        nc.tensor.matmul(
                            pAff[:, 512:1024],
                            mT8[:, 0, kp],
                            tt8_sb[:, 0, kp],
                            start=(kp == 0),
                            stop=(kp == 1),
                            perf_mode=DR,
                        )
                    emit_class(len(class_ops))
                    nc.scalar.activation(
                        jEt[:, :],
                        pAff[:, 512:1024],
                        Exp,
                        scale=1.0 / BOOST,
                        accum_out=stage[0:64, 5:6],
                    )
                    nc.vector.tensor_scalar_mul(
                        o3b[:, 512:1024], pAff[:, 512:1024], 1.0 / BOOST
                    )

                    # s-direction: needs txt means
                    emit_means(1)
                    for kp in range(2):
                        nc.tensor.matmul(
                            pAff[:, 0:512],
                            mT8[:, 1, kp],
                            is8_sb[:, kp],
                            start=(kp == 0),
                            stop=(kp == 1),
                            perf_mode=DR,
                        )
                    nc.scalar.activation(
                        Esc[:, :], pAff[:, 0:512], Exp, scale=1.0 / BOOST
                    )
                    nc.vector.tensor_scalar_mul(
                        o3b[:, 0:512], pAff[:, 0:512], 1.0 / BOOST
                    )
                    nc.gpsimd.dma_start(out3[:], o3b[:])
                    # host computes zs = sum_c cnt_c * exp(s)[c, i]
                    nc.gpsimd.dma_start(out4[:], Esc[:, :])

            nc.gpsimd.dma_start(out[:], stage[:])

    nc.compile()
    return nc


def _combine(outs, outzs, outcs, out3s, out4s, outps, cnt, label):
    o = np.stack([np.asarray(x, dtype=np.float64) for x in outs])  # [8,128,8]
    oz = np.stack([np.asarray(x, dtype=np.float64) for x in outzs])  # [8,128,4,4]
    oc = np.stack([np.asarray(x, dtype=np.float64) for x in outcs])  # [8,128,4,1024]
    o3 = np.stack([np.asarray(x, dtype=np.float64) for x in out3s])  # [8,64,1024]
    o4 = np.stack([np.asarray(x, dtype=np.float64) for x in out4s])  # [8,64,512]
    op = np.stack([np.asarray(x, dtype=np.float64)[0] for x in outps])  # [8,512]

    G = o[:, 0, 4]  # [8]
    diag = op.reshape(B)  # core-major, i-order
    zs = (cnt.astype(np.float64)[None, :, None] * o4).sum(axis=1).reshape(B)
    # column-sum partials: collapse partitions, unrotate windows
    cs_rot = oc.sum(axis=1).reshape(N_CORES, B)  # [8, 4096] rotated cols
    cs = np.empty((N_CORES, B))
    for c in range(N_CORES):
        for w in range(NW):
            r = (w - c) % NW
            cs[c, 512 * w : 512 * (w + 1)] = cs_rot[c, 512 * r : 512 * (r + 1)]
    # row sums: slot t=1 is a direct ACT sum; slots 0/2/3 are DVE running
    # sums of the colacc (R0=A0, R1=Z1, R2=A2-A0-Z1, R3=A3-A2)
    R = np.empty_like(oz)  # [8, 128, t, g]
    R[:, :, 0, :] = oz[:, :, 0, :]
    R[:, :, 1, :] = oz[:, :, 1, :]
    R[:, :, 2, :] = oz[:, :, 2, :] - oz[:, :, 0, :] - oz[:, :, 1, :]
    R[:, :, 3, :] = oz[:, :, 3, :] - oz[:, :, 2, :]
    rowsums = np.empty(B)
    for c in range(N_CORES):
        for t in range(RT):
            rows = slice(SHARD * c + 128 * t, SHARD * c + 128 * (t + 1))
            rowsums[rows] = R[c, :, t, :].sum(axis=1)
    # bf16 exp underflows below ~e^-88; a fully-underflowed row's true
    # LSE sits at the clamp point, so clamping costs <~1 logit there.
    rowsums = np.maximum(rowsums, 1e-38)
    lse1 = np.repeat(G, SHARD) + np.log(rowsums)
    Mg = G.max()
    colsums = np.maximum((cs * np.exp(G - Mg)[:, None]).sum(axis=0), 1e-38)
    lse2 = Mg + np.log(colsums)  # [B]
    loss_i2t = -np.mean(diag - lse1)
    loss_t2i = -np.mean(diag - lse2)
    contr = 0.5 * (loss_i2t + loss_t2i)

    lab = np.asarray(label, dtype=np.int64)
    sT = o3[:, :, 0:512]  # [8, 64, 512]
    tT = o3[:, :, 512:1024]
    tsums = o[:, 0:64, 5]  # [8, 64]
    idx_core = np.arange(B) // SHARD
    idx_i = np.arange(B) % SHARD
    sdiag = sT[idx_core, lab, idx_i]
    tvals = tT[idx_core, lab, idx_i]
    alse = np.log(zs)
    a_i2t = -np.mean(sdiag - alse)
    collse = np.log(tsums.sum(axis=0))  # [64]
    a_t2i = -np.mean(tvals - collse[lab])
    affil = 0.5 * (a_i2t + a_t2i)
    return np.float32(contr + affil)


def kernel(image_feat, text_feat, label, temp, temp2):
    global LAST_RESULTS
    img = np.ascontiguousarray(np.asarray(image_feat, dtype=np.float32))
    txt = np.ascontiguousarray(np.asarray(text_feat, dtype=np.float32))
    labv = np.asarray(label).astype(np.int64).reshape(B)
    tv = float(np.asarray(temp))
    t2v = float(np.asarray(temp2))

    nc = _compiled(tv, t2v)

    import ml_dtypes

    f8 = ml_dtypes.float8_e4m3
    rt_st = float(np.sqrt(1.0 / tv))
    # full text transposed fp8, window-major: [p, w, kp, ki, j]
    ttw = np.ascontiguousarray(
        (txt.T * rt_st)
        .astype(f8)
        .reshape(2, 2, 128, NW, 512)
        .transpose(2, 3, 0, 1, 4)
    )  # [128, 8, 2, 2, 512]

    def _pmT8(x):
        # [512, D] shard -> [p, kp, ki, i] fp8 transposed layout
        return np.ascontiguousarray(
            x.T.reshape(2, 2, 128, SHARD).transpose(2, 0, 1, 3)
        )

    def _nat8(x):
        # [B, D] full -> [p, op, oi, d] fp8 natural layout
        return np.ascontiguousarray(
            x.reshape(NOP, 2, 128, D).transpose(2, 0, 1, 3)
        )

    nat8_in = np.ascontiguousarray(
        np.stack([_nat8(img.astype(f8)), _nat8(txt.astype(f8))], axis=1)
    )
    # one-hot [j, c] in the same j-partition layout as nat8
    ohf = (labv[:, None] == np.arange(NCLS)[None, :]).astype(f8)  # [B, 64]
    oh8_in = np.ascontiguousarray(
        ohf.reshape(NOP, 2, 128, NCLS).transpose(2, 0, 1, 3)
    )
    cnt = np.bincount(labv, minlength=NCLS).astype(np.float32)  # [64]
    aux_in = np.ascontiguousarray(
        np.stack([1.0 / np.maximum(cnt, 1.0), cnt], axis=1)
    )  # [64, 2]

    in_maps = []
    for c in range(N_CORES):
        sl = slice(SHARD * c, SHARD * (c + 1))
        in_maps.append(
            {
                "aux": aux_in,
                "is8": _pmT8((img[sl] * rt_st).astype(f8)),
                "tt8": np.ascontiguousarray(np.roll(ttw, -c, axis=1)),
                "oh8": oh8_in,
                "nat8": nat8_in,
            }
        )

    from concourse import bass_utils

    res = bass_utils.run_bass_kernel_spmd(
        nc, in_maps, core_ids=list(range(N_CORES))
    )
    LAST_RESULTS = res
    return _combine(
        [r["out"] for r in res.results],
        [r["outz"] for r in res.results],
        [r["outc"] for r in res.results],
        [r["out3"] for r in res.results],
        [r["out4"] for r in res.results],
        [r["outp"] for r in res.results],
        cnt,
        labv,
    )


# revision 28
# speedup vs baseline: 1.3733x; 1.3733x over previous
"""Trainium2 Bass kernel for nn_HarMABase contrastive+affiliation loss (v5).

B=4096, D=512, N_CLASSES=64, 8 NeuronCores, data-parallel over batch rows.

Per core c (rows r = 512c..512c+512), all feature math in fp8 DoubleRow:
  - tt8 = full text.T, fp8, in 512-col WINDOWS rotated per core so the
    core's own 512-col window is window 0; host unrotates column sums.
  - input DMAs dispatch from FOUR engines in parallel (sync carries the
    big streams in priority order tt8 -> nat8-img -> nat8-txt); outputs
    dispatch from the idle gpsimd engine.
  - main loop: 4 col groups of 1024 (2 windows), psum [128,1024]x3 bufs;
    exp on ACT with per-core shift G (max of a 256-col sample + 6); the
    colacc add on DVE carries accum_out running row sums (A[t,g]); the
    host recovers per-tile row sums by differencing A along t.  colacc
    ([128,1024] bf16 column-sum partials) exports RAW per group; host
    collapses partitions.
  - class sums (host-shipped one-hot fp8 x natural features fp8) into a
    2-bank psum alive all kernel; img half interleaved 2-ops-per-tile
    into groups 2-3, txt half 2-ops-per-tile in group 3 + post-loop.
  - affil split per direction and pipelined: as soon as the img class
    half lands, means->transpose->t.T matmul->exp overlap the rest of
    the main loop tail; txt direction follows.  Means scaled by
    8/(temp2*sqrt(1/temp))/cnt, cast fp8 (x8 boost keeps fp8 normals,
    undone by exp scale=1/8); raw s.T/t.T export bf16; zs collapse
    (cnt-weighted) happens on host from exp(s.T).
Host combines per-row/per-class partials into the scalar loss in float64.
"""

import functools
import os
import sys

import numpy as np

for _p in ("/root/.axon_site", "/root/.axon_site/_ro/trn_rl_repo"):
    if os.path.isdir(_p) and _p not in sys.path:
        sys.path.insert(0, _p)
if not os.path.isdir("/root/.axon_site/_ro/trn_rl_repo") and os.path.isdir(
    "/opt/trn_rl_repo"
):
    if "/opt/trn_rl_repo" not in sys.path:
        sys.path.insert(0, "/opt/trn_rl_repo")

N_CORES = 8
B = 4096
D = 512
NCLS = 64
SHARD = B // N_CORES  # 512
RT = SHARD // 128  # 4 row tiles per core
NT = B // 128  # 32 row tiles in the full batch
NOP = NT // 2  # 16 row-tile PAIRS (DoubleRow)
NW = B // 512  # 8 column windows of 512
GCH = 1024  # columns per psum group (2 banks)
NG = B // GCH  # 4 groups
LAST_RESULTS = None


@functools.lru_cache(maxsize=4)
def _compiled(temp: float, temp2: float):
    import concourse.bass as bass  # noqa: F401
    import concourse.tile as tile
    import concourse.bass_isa as bass_isa
    from concourse import bacc, mybir
    from concourse.masks import make_identity

    f32 = mybir.dt.float32
    bf16 = mybir.dt.bfloat16
    fp8 = mybir.dt.float8e4
    Exp = mybir.ActivationFunctionType.Exp
    X = mybir.AxisListType.X
    ALU = mybir.AluOpType
    DR = mybir.MatmulPerfMode.DoubleRow

    # fp8 operands arrive host-prescaled by sqrt(1/temp); means get an
    # extra x8 boost into fp8's normal range, undone by exp scale=1/8.
    BOOST = 8.0
    rt_st = float(np.sqrt(1.0 / temp))
    mean_imm = BOOST / (temp2 * rt_st)

    nc = bacc.Bacc(
        "TRN2",
        target_bir_lowering=False,
        debug=False,
        num_devices=N_CORES,
    )

    aux = nc.dram_tensor("aux", [64, 2], f32, kind="ExternalInput")
    is8 = nc.dram_tensor("is8", [128, 2, 2, SHARD], fp8, kind="ExternalInput")
    tt8 = nc.dram_tensor("tt8", [128, NW, 2, 2, 512], fp8, kind="ExternalInput")
    oh8 = nc.dram_tensor("oh8", [128, NOP, 2, NCLS], fp8, kind="ExternalInput")
    nat8 = nc.dram_tensor("nat8", [128, 2, NOP, 2, D], fp8, kind="ExternalInput")
    out = nc.dram_tensor("out", [128, 8], f32, kind="ExternalOutput")
    outz = nc.dram_tensor("outz", [128, RT, NG], f32, kind="ExternalOutput")
    outc = nc.dram_tensor("outc", [128, NG, GCH], bf16, kind="ExternalOutput")
    out3 = nc.dram_tensor("out3", [64, 1024], bf16, kind="ExternalOutput")
    out4 = nc.dram_tensor("out4", [64, 512], bf16, kind="ExternalOutput")
    outp = nc.dram_tensor("outp", [1, 512], f32, kind="ExternalOutput")

    with tile.TileContext(nc) as tc:
        with (
            tc.tile_pool(name="const", bufs=1) as const,
            tc.tile_pool(name="junk", bufs=3) as junkp,
            tc.tile_pool(name="colac", bufs=2) as colaccp,
            tc.tile_pool(name="stats", bufs=1) as statp,
        ):
            # ---------- input loads: parallel dispatch, then priority ----
            tt8_sb = const.tile([128, NW, 2, 2, 512], fp8, tag="tt8")
            is8_sb = const.tile([128, 2, 2, SHARD], fp8, tag="is8")
            aux_sb = const.tile([64, 2], f32, tag="aux")
            oh8_sb = const.tile([128, NOP, 2, NCLS], fp8, tag="oh8")
            nat8_sb = const.tile([128, 2, NOP, 2, D], fp8, tag="nat8")
            # phase A: first windows + small tensors, uncontended; later
            # phases gated by sync-engine drains so the DMA queues'
            # round-robin fair-sharing can't starve the critical path.
            nc.sync.dma_start(tt8_sb[:, 0:2], tt8[:, 0:2])
            nc.gpsimd.dma_start(is8_sb[:], is8[:, :, :, :])
            nc.gpsimd.dma_start(aux_sb[:], aux[:, :])
            nc.scalar.dma_start(oh8_sb[:], oh8[:, :, :, :])
            nc.sync.drain()
            nc.sync.dma_start(tt8_sb[:, 2:4], tt8[:, 2:4])
            nc.sync.dma_start(tt8_sb[:, 4:6], tt8[:, 4:6])
            nc.sync.dma_start(tt8_sb[:, 6:8], tt8[:, 6:8])
            nc.sync.drain()
            nc.sync.dma_start(nat8_sb[:, 0], nat8[:, 0])
            nc.sync.dma_start(nat8_sb[:, 1], nat8[:, 1])

            # ---------- constants / small setup ----------
            # gpsimd runs exactly ONE ISA op family (make_identity's
            # affine_select): more families would thrash multi-MB library
            # loads that starve against input DMA traffic.
            stage = const.tile([128, 8], f32, tag="stage")
            nc.vector.memset(stage[:], 0.0)
            ident = const.tile([128, 128], f32, tag="ident")
            make_identity(nc, ident[:])
            ones1 = const.tile([128, 1], bf16, tag="ones1")
            nc.vector.memset(ones1[:], 1.0)
            onesr = const.tile([1, 128], f32, tag="onesr")
            nc.vector.memset(onesr[:], 1.0)
            gmx = statp.tile([1, 1], f32, tag="gmx")
            pdg_sb = statp.tile([1, 512], f32, tag="pdg_sb")

            Gt = statp.tile([128, 2], f32, tag="Gt")  # col0 G, col1 -G
            zbG = statp.tile([128, RT, NG], f32, tag="zbG")

            with tc.tile_pool(name="psAux", bufs=1, space="PSUM") as psAux:
                # 2 banks, alive all kernel: diag [1,512] + class sums
                # [64,1024] -> transposed means (after each half retires).
                pcls = psAux.tile([128, 1024], f32, tag="pcls", name="pcls")

                d4 = junkp.tile([128, 2, 2, SHARD], bf16, tag="d4", name="d4")

                # class-sum ops, spread across late main-loop row tiles:
                # img (co=0) during groups 2-3, txt (co=512) g3 + post.
                class_ops = [(0, op) for op in range(NOP)] + [
                    (512, op) for op in range(NOP)
                ]

                def emit_class(n):
                    while n > 0 and class_ops:
                        co, op = class_ops.pop(0)
                        src = nat8_sb[:, 0] if co == 0 else nat8_sb[:, 1]
                        nc.tensor.matmul(
                            pcls[0:64, co : co + 512],
                            oh8_sb[:, op, :, :],
                            src[:, op, :, :],
                            start=(op == 0),
                            stop=(op == NOP - 1),
                            perf_mode=DR,
                        )
                        n -= 1

                mFb = statp.tile([64, 1024], f32, tag="mFb")
                mT8 = statp.tile([128, 2, 2, 2, NCLS], fp8, tag="mT8")

                def emit_means(di):
                    # means for direction di (0=img,1=txt) + transpose +
                    # fp8 cast into mT8[:, di]; pm reuses the retired
                    # class psum half.
                    co = 512 * di
                    nc.vector.scalar_tensor_tensor(
                        out=mFb[:, co : co + 512],
                        in0=pcls[0:64, co : co + 512],
                        scalar=mean_imm,
                        in1=aux_sb[:, 0:1].broadcast_to([64, 512]),
                        op0=ALU.mult,
                        op1=ALU.mult,
                    )
                    pm = pcls[:, co : co + 256]
                    for c4 in range(4):
                        nc.tensor.transpose(
                            pm[:, 64 * c4 : 64 * (c4 + 1)],
                            mFb[:, co + 128 * c4 : co + 128 * (c4 + 1)],
                            ident[0:64, 0:64],
                        )
                    nc.vector.tensor_copy(
                        mT8[:, di].rearrange("p b c d -> p (b c d)"), pm[:]
                    )

                # PE warmup during the DMA wait: junk matmuls ramp the
                # pstate so the first real matmuls run at full clock.
                # ones1-based: no dependency on gpsimd's identity build.
                for w in range(22):
                    nc.tensor.matmul(
                        pcls[0:1, 512:513],
                        ones1[:, 0:1],
                        ones1[:, 0:1],
                        start=True,
                        stop=True,
                    )

                with tc.tile_pool(name="psumB", bufs=3, space="PSUM") as psumB:
                    for g in range(NG):
                        colacc = colaccp.tile(
                            [128, GCH], bf16, tag="colacc", name="colacc"
                        )
                        for t in range(RT):
                            ps = psumB.tile([128, GCH], f32, tag="mm", name="ps")
                            for j in range(2):
                                for kp in range(2):
                                    nc.tensor.matmul(
                                        ps[:, 512 * j : 512 * (j + 1)],
                                        is8_sb[:, kp, :, 128 * t : 128 * (t + 1)],
                                        tt8_sb[:, 2 * g + j, kp, :, :],
                                        start=(kp == 0),
                                        stop=(kp == 1),
                                        perf_mode=DR,
                                    )
                                if g == 0 and t == 0 and j == 0:
                                    # G = max(256-col sample) + 6; cross-
                                    # partition max via PE transpose +
                                    # ones-broadcast (no gpsimd library).
                                    nc.vector.tensor_reduce(
                                        Gt[:, 0:1], ps[:, 0:256], axis=X, op=ALU.max
                                    )
                                    nc.vector.tensor_scalar_add(
                                        Gt[:, 0:1], Gt[:, 0:1], 6.0
                                    )
                            if g == 0 and t == 0:
                                nc.tensor.transpose(
                                    pcls[0:1, 896:1024], Gt[:, 0:1], ident[:, :]
                                )
                                nc.vector.tensor_reduce(
                                    gmx[:, :], pcls[0:1, 896:1024], axis=X, op=ALU.max
                                )
                                nc.tensor.matmul(
                                    pcls[:, 895:896],
                                    onesr[:, :],
                                    gmx[:, :],
                                    start=True,
                                    stop=True,
                                )
                                nc.vector.tensor_copy(Gt[:, 0:1], pcls[:, 895:896])
                                nc.vector.tensor_scalar_mul(
                                    Gt[:, 1:2], Gt[:, 0:1], -1.0
                                )
                                nc.vector.tensor_copy(stage[:, 4:5], Gt[:, 0:1])
                            if g == 1 and t in (2, 3):
                                # diag ones-matmuls; must finish before the
                                # class img half resets pcls[:, 0:512]
                                for k in (0, 1) if t == 2 else (2, 3):
                                    nc.tensor.matmul(
                                        pcls[0:1, 0:512],
                                        ones1[:, 0:1],
                                        d4[:, k // 2, k % 2, :],
                                        start=(k == 0),
                                        stop=(k == 3),
                                    )
                                if t == 3:
                                    nc.vector.tensor_copy(
                                        pdg_sb[:], pcls[0:1, 0:512]
                                    )
                                    nc.gpsimd.dma_start(outp[:, :], pdg_sb[:])
                            jk = junkp.tile([128, GCH], bf16, tag="jexp", name="jexp")
                            # rowsums: t==1 via ACT accumulator, others via
                            # DVE accumulators on the colacc op (running
                            # sums; host differences along t).
                            if t == 1:
                                nc.scalar.activation(
                                    jk[:],
                                    ps[:],
                                    Exp,
                                    bias=Gt[:, 1:2],
                                    accum_out=zbG[:, t, g : g + 1],
                                )
                                nc.vector.tensor_tensor(
                                    colacc[:], colacc[:], jk[:], op=ALU.add
                                )
                            elif t == 0:
                                nc.scalar.activation(jk[:], ps[:], Exp, bias=Gt[:, 1:2])
                                nc.vector.tensor_scalar(
                                    colacc[:],
                                    jk[:],
                                    1.0,
                                    0.0,
                                    op0=ALU.mult,
                                    op1=ALU.add,
                                    accum_out=zbG[:, t, g : g + 1],
                                )
                            else:
                                nc.scalar.activation(jk[:], ps[:], Exp, bias=Gt[:, 1:2])
                                nc.vector.scalar_tensor_tensor(
                                    out=colacc[:],
                                    in0=jk[:],
                                    scalar=1.0,
                                    in1=colacc[:],
                                    op0=ALU.mult,
                                    op1=ALU.add,
                                    accum_out=zbG[:, t, g : g + 1],
                                )
                            if g == 0:
                                # diag products, chunked across g0's DVE slack
                                nc.vector.tensor_tensor(
                                    d4[:, t // 2, t % 2, :],
                                    is8_sb[:, t // 2, t % 2, :],
                                    tt8_sb[:, 0, t // 2, t % 2, :],
                                    op=ALU.mult,
                                )
                        nc.gpsimd.dma_start(outc[:, g], colacc[:])
                    emit_class(16)
                    nc.gpsimd.dma_start(outz[:], zbG[:])

                # ---------- affil, per-direction pipeline ----------
                with tc.tile_pool(name="psA", bufs=1, space="PSUM") as psA:
                    pAff = psA.tile([64, 2 * SHARD], f32, tag="pAff", name="pAff")
                    o3b = statp.tile([64, 1024], bf16, tag="o3b")
                    Esc = junkp.tile([64, SHARD], bf16, tag="Esc", name="Esc")
                    jEt = junkp.tile([64, SHARD], bf16, tag="jEt", name="jEt")

                    # t-direction: needs img means (class img half done at
                    # end of g3 interleave)
                    emit_means(0)
                    emit_class(4)
                    for kp in range(2):
                        nc.tensor.matmul(
                            pAff[:, 512:1024],
                            mT8[:, 0, kp],
                            tt8_sb[:, 0, kp],
                            start=(kp == 0),
                            stop=(kp == 1),
                            perf_mode=DR,
                        )
                    emit_class(len(class_ops))
                    nc.scalar.activation(
                        jEt[:, :],
                        pAff[:, 512:1024],
                        Exp,
                        scale=1.0 / BOOST,
                        accum_out=stage[0:64, 5:6],
                    )
                    nc.vector.tensor_scalar_mul(
                        o3b[:, 512:1024], pAff[:, 512:1024], 1.0 / BOOST
                    )

                    # s-direction: needs txt means
                    emit_means(1)
                    for kp in range(2):
                        nc.tensor.matmul(
                            pAff[:, 0:512],
                            mT8[:, 1, kp],
                            is8_sb[:, kp],
                            start=(kp == 0),
                            stop=(kp == 1),
                            perf_mode=DR,
                        )
                    nc.scalar.activation(
                        Esc[:, :], pAff[:, 0:512], Exp, scale=1.0 / BOOST
                    )
                    nc.vector.tensor_scalar_mul(
                        o3b[:, 0:512], pAff[:, 0:512], 1.0 / BOOST
                    )
                    nc.gpsimd.dma_start(out3[:], o3b[:])
                    # host computes zs = sum_c cnt_c * exp(s)[c, i]
                    nc.gpsimd.dma_start(out4[:], Esc[:, :])

            nc.gpsimd.dma_start(out[:], stage[:])

    nc.compile()
    return nc


def _combine(outs, outzs, outcs, out3s, out4s, outps, cnt, label):
    o = np.stack([np.asarray(x, dtype=np.float64) for x in outs])  # [8,128,8]
    oz = np.stack([np.asarray(x, dtype=np.float64) for x in outzs])  # [8,128,4,4]
    oc = np.stack([np.asarray(x, dtype=np.float64) for x in outcs])  # [8,128,4,1024]
    o3 = np.stack([np.asarray(x, dtype=np.float64) for x in out3s])  # [8,64,1024]
    o4 = np.stack([np.asarray(x, dtype=np.float64) for x in out4s])  # [8,64,512]
    op = np.stack([np.asarray(x, dtype=np.float64)[0] for x in outps])  # [8,512]

    G = o[:, 0, 4]  # [8]
    diag = op.reshape(B)  # core-major, i-order
    zs = (cnt.astype(np.float64)[None, :, None] * o4).sum(axis=1).reshape(B)
    # column-sum partials: collapse partitions, unrotate windows
    cs_rot = oc.sum(axis=1).reshape(N_CORES, B)  # [8, 4096] rotated cols
    cs = np.empty((N_CORES, B))
    for c in range(N_CORES):
        for w in range(NW):
            r = (w - c) % NW
            cs[c, 512 * w : 512 * (w + 1)] = cs_rot[c, 512 * r : 512 * (r + 1)]
    # row sums: slot t=1 is a direct ACT sum; slots 0/2/3 are DVE running
    # sums of the colacc (R0=A0, R1=Z1, R2=A2-A0-Z1, R3=A3-A2)
    R = np.empty_like(oz)  # [8, 128, t, g]
    R[:, :, 0, :] = oz[:, :, 0, :]
    R[:, :, 1, :] = oz[:, :, 1, :]
    R[:, :, 2, :] = oz[:, :, 2, :] - oz[:, :, 0, :] - oz[:, :, 1, :]
    R[:, :, 3, :] = oz[:, :, 3, :] - oz[:, :, 2, :]
    rowsums = np.empty(B)
    for c in range(N_CORES):
        for t in range(RT):
            rows = slice(SHARD * c + 128 * t, SHARD * c + 128 * (t + 1))
            rowsums[rows] = R[c, :, t, :].sum(axis=1)
    # bf16 exp underflows below ~e^-88; a fully-underflowed row's true
    # LSE sits at the clamp point, so clamping costs <~1 logit there.
    rowsums = np.maximum(rowsums, 1e-38)
    lse1 = np.repeat(G, SHARD) + np.log(rowsums)
    Mg = G.max()
    colsums = np.maximum((cs * np.exp(G - Mg)[:, None]).sum(axis=0), 1e-38)
    lse2 = Mg + np.log(colsums)  # [B]
    loss_i2t = -np.mean(diag - lse1)
    loss_t2i = -np.mean(diag - lse2)
    contr = 0.5 * (loss_i2t + loss_t2i)

    lab = np.asarray(label, dtype=np.int64)
    sT = o3[:, :, 0:512]  # [8, 64, 512]
    tT = o3[:, :, 512:1024]
    tsums = o[:, 0:64, 5]  # [8, 64]
    idx_core = np.arange(B) // SHARD
    idx_i = np.arange(B) % SHARD
    sdiag = sT[idx_core, lab, idx_i]
    tvals = tT[idx_core, lab, idx_i]
    alse = np.log(zs)
    a_i2t = -np.mean(sdiag - alse)
    collse = np.log(tsums.sum(axis=0))  # [64]
    a_t2i = -np.mean(tvals - collse[lab])
    affil = 0.5 * (a_i2t + a_t2i)
    return np.float32(contr + affil)


def kernel(image_feat, text_feat, label, temp, temp2):
    global LAST_RESULTS
    img = np.ascontiguousarray(np.asarray(image_feat, dtype=np.float32))
    txt = np.ascontiguousarray(np.asarray(text_feat, dtype=np.float32))
    labv = np.asarray(label).astype(np.int64).reshape(B)
    tv = float(np.asarray(temp))
    t2v = float(np.asarray(temp2))

    nc = _compiled(tv, t2v)

    import ml_dtypes

    f8 = ml_dtypes.float8_e4m3
    rt_st = float(np.sqrt(1.0 / tv))
    # full text transposed fp8, window-major: [p, w, kp, ki, j]
    ttw = np.ascontiguousarray(
        (txt.T * rt_st)
        .astype(f8)
        .reshape(2, 2, 128, NW, 512)
        .transpose(2, 3, 0, 1, 4)
    )  # [128, 8, 2, 2, 512]

    def _pmT8(x):
        # [512, D] shard -> [p, kp, ki, i] fp8 transposed layout
        return np.ascontiguousarray(
            x.T.reshape(2, 2, 128, SHARD).transpose(2, 0, 1, 3)
        )

    def _nat8(x):
        # [B, D] full -> [p, op, oi, d] fp8 natural layout
        return np.ascontiguousarray(
            x.reshape(NOP, 2, 128, D).transpose(2, 0, 1, 3)
        )

    nat8_in = np.ascontiguousarray(
        np.stack([_nat8(img.astype(f8)), _nat8(txt.astype(f8))], axis=1)
    )
    # one-hot [j, c] in the same j-partition layout as nat8
    ohf = (labv[:, None] == np.arange(NCLS)[None, :]).astype(f8)  # [B, 64]
    oh8_in = np.ascontiguousarray(
        ohf.reshape(NOP, 2, 128, NCLS).transpose(2, 0, 1, 3)
    )
    cnt = np.bincount(labv, minlength=NCLS).astype(np.float32)  # [64]
    aux_in = np.ascontiguousarray(
        np.stack([1.0 / np.maximum(cnt, 1.0), cnt], axis=1)
    )  # [64, 2]

    in_maps = []
    for c in range(N_CORES):
        sl = slice(SHARD * c, SHARD * (c + 1))
        in_maps.append(
            {
                "aux": aux_in,
                "is8": _pmT8((img[sl] * rt_st).astype(f8)),
                "tt8": np.ascontiguousarray(np.roll(ttw, -c, axis=1)),
                "oh8": oh8_in,
                "nat8": nat8_in,
            }
        )

    from concourse import bass_utils

    res = bass_utils.run_bass_kernel_spmd(
        nc, in_maps, core_ids=list(range(N_CORES))
    )
    LAST_RESULTS = res
    return _combine(
        [r["out"] for r in res.results],
        [r["outz"] for r in res.results],
        [r["outc"] for r in res.results],
        [r["out3"] for r in res.results],
        [r["out4"] for r in res.results],
        [r["outp"] for r in res.results],
        cnt,
        labv,
    )


# revision 33
# speedup vs baseline: 1.4213x; 1.0349x over previous
"""Trainium2 Bass kernel for nn_HarMABase contrastive+affiliation loss (v5).

B=4096, D=512, N_CLASSES=64, 8 NeuronCores, data-parallel over batch rows.

Per core c (rows r = 512c..512c+512), all feature math in fp8 DoubleRow:
  - tt8 = full text.T, fp8, in 512-col WINDOWS rotated per core so the
    core's own 512-col window is window 0; host unrotates column sums.
  - input DMAs dispatch from FOUR engines in parallel (sync carries the
    big streams in priority order tt8 -> nat8-img -> nat8-txt); outputs
    dispatch from the idle gpsimd engine.
  - main loop: 4 col groups of 1024 (2 windows), psum [128,1024]x3 bufs;
    exp on ACT with per-core shift G (max of a 256-col sample + 6); the
    colacc add on DVE carries accum_out running row sums (A[t,g]); the
    host recovers per-tile row sums by differencing A along t.  colacc
    ([128,1024] bf16 column-sum partials) exports RAW per group; host
    collapses partitions.
  - class sums (host-shipped one-hot fp8 x natural features fp8) into a
    2-bank psum alive all kernel; img half interleaved 2-ops-per-tile
    into groups 2-3, txt half 2-ops-per-tile in group 3 + post-loop.
  - affil split per direction and pipelined: as soon as the img class
    half lands, means->transpose->t.T matmul->exp overlap the rest of
    the main loop tail; txt direction follows.  Means scaled by
    8/(temp2*sqrt(1/temp))/cnt, cast fp8 (x8 boost keeps fp8 normals,
    undone by exp scale=1/8); raw s.T/t.T export bf16; zs collapse
    (cnt-weighted) happens on host from exp(s.T).
Host combines per-row/per-class partials into the scalar loss in float64.
"""

import functools
import os
import sys

import numpy as np

for _p in ("/root/.axon_site", "/root/.axon_site/_ro/trn_rl_repo"):
    if os.path.isdir(_p) and _p not in sys.path:
        sys.path.insert(0, _p)
if not os.path.isdir("/root/.axon_site/_ro/trn_rl_repo") and os.path.isdir(
    "/opt/trn_rl_repo"
):
    if "/opt/trn_rl_repo" not in sys.path:
        sys.path.insert(0, "/opt/trn_rl_repo")

N_CORES = 8
B = 4096
D = 512
NCLS = 64
SHARD = B // N_CORES  # 512
RT = SHARD // 128  # 4 row tiles per core
NT = B // 128  # 32 row tiles in the full batch
NOP = NT // 2  # 16 row-tile PAIRS (DoubleRow)
NW = B // 512  # 8 column windows of 512
GCH = 1024  # columns per psum group (2 banks)
NG = B // GCH  # 4 groups
LAST_RESULTS = None


@functools.lru_cache(maxsize=4)
def _compiled(temp: float, temp2: float):
    import concourse.bass as bass  # noqa: F401
    import concourse.tile as tile
    import concourse.bass_isa as bass_isa
    from concourse import bacc, mybir
    from concourse.masks import make_identity

    f32 = mybir.dt.float32
    bf16 = mybir.dt.bfloat16
    fp8 = mybir.dt.float8e4
    Exp = mybir.ActivationFunctionType.Exp
    X = mybir.AxisListType.X
    ALU = mybir.AluOpType
    DR = mybir.MatmulPerfMode.DoubleRow

    # fp8 operands arrive host-prescaled by sqrt(1/temp); means get an
    # extra x8 boost into fp8's normal range, undone by exp scale=1/8.
    BOOST = 8.0
    rt_st = float(np.sqrt(1.0 / temp))
    mean_imm = BOOST / (temp2 * rt_st)

    nc = bacc.Bacc(
        "TRN2",
        target_bir_lowering=False,
        debug=False,
        num_devices=N_CORES,
    )

    aux = nc.dram_tensor("aux", [64, 2], f32, kind="ExternalInput")
    is8 = nc.dram_tensor("is8", [128, 2, 2, SHARD], fp8, kind="ExternalInput")
    tt8 = nc.dram_tensor("tt8", [128, NW, 2, 2, 512], fp8, kind="ExternalInput")
    oh8 = nc.dram_tensor("oh8", [128, NOP, 2, NCLS], fp8, kind="ExternalInput")
    nat8 = nc.dram_tensor("nat8", [128, 2, NOP, 2, D], fp8, kind="ExternalInput")
    out = nc.dram_tensor("out", [128, 8], f32, kind="ExternalOutput")
    outz = nc.dram_tensor("outz", [128, RT, NG], f32, kind="ExternalOutput")
    outc = nc.dram_tensor("outc", [128, NG, GCH], bf16, kind="ExternalOutput")
    out3 = nc.dram_tensor("out3", [64, 1024], bf16, kind="ExternalOutput")
    out4 = nc.dram_tensor("out4", [64, 512], bf16, kind="ExternalOutput")
    outp = nc.dram_tensor("outp", [1, 512], f32, kind="ExternalOutput")

    with tile.TileContext(nc) as tc:
        with (
            tc.tile_pool(name="const", bufs=1) as const,
            tc.tile_pool(name="junk", bufs=3) as junkp,
            tc.tile_pool(name="colac", bufs=2) as colaccp,
            tc.tile_pool(name="stats", bufs=1) as statp,
        ):
            # ---------- input loads: parallel dispatch, then priority ----
            tt8_sb = const.tile([128, NW, 2, 2, 512], fp8, tag="tt8")
            is8_sb = const.tile([128, 2, 2, SHARD], fp8, tag="is8")
            aux_sb = const.tile([64, 2], f32, tag="aux")
            oh8_sb = const.tile([128, NOP, 2, NCLS], fp8, tag="oh8")
            nat8_sb = const.tile([128, 2, NOP, 2, D], fp8, tag="nat8")
            # phase A: first windows + small tensors, uncontended; later
            # phases gated by sync-engine drains so the DMA queues'
            # round-robin fair-sharing can't starve the critical path.
            nc.sync.dma_start(tt8_sb[:, 0:2], tt8[:, 0:2])
            nc.sync.dma_start(is8_sb[:], is8[:, :, :, :])
            nc.sync.dma_start(aux_sb[:], aux[:, :])
            nc.scalar.dma_start(oh8_sb[:], oh8[:, :, :, :])
            nc.sync.drain()
            nc.sync.dma_start(tt8_sb[:, 2:4], tt8[:, 2:4])
            nc.sync.dma_start(tt8_sb[:, 4:6], tt8[:, 4:6])
            nc.sync.dma_start(tt8_sb[:, 6:8], tt8[:, 6:8])
            nc.sync.drain()
            nc.sync.dma_start(nat8_sb[:, 1], nat8[:, 1])
            nc.sync.dma_start(nat8_sb[:, 0], nat8[:, 0])

            # ---------- constants / small setup ----------
            # gpsimd runs exactly ONE ISA op family (make_identity's
            # affine_select): more families would thrash multi-MB library
            # loads that starve against input DMA traffic.
            stage = const.tile([128, 8], f32, tag="stage")
            nc.vector.memset(stage[:], 0.0)
            ident = const.tile([128, 128], f32, tag="ident")
            make_identity(nc, ident[:])
            ones1 = const.tile([128, 1], bf16, tag="ones1")
            nc.vector.memset(ones1[:], 1.0)
            onesr = const.tile([1, 128], bf16, tag="onesr")
            nc.vector.memset(onesr[:], 1.0)
            gmx = statp.tile([1, 1], bf16, tag="gmx")
            pdg_sb = statp.tile([1, 512], f32, tag="pdg_sb")

            Gt = statp.tile([128, 2], f32, tag="Gt")  # col0 G, col1 -G
            zbG = statp.tile([128, RT, NG], f32, tag="zbG")

            with tc.tile_pool(name="psAux", bufs=1, space="PSUM") as psAux:
                # 2 banks, alive all kernel: diag [1,512] + class sums
                # [64,1024] -> transposed means (after each half retires).
                pcls = psAux.tile([128, 1024], f32, tag="pcls", name="pcls")

                d4 = junkp.tile([128, 2, 2, SHARD], bf16, tag="d4", name="d4")

                # class-sum ops: txt half first (the s-direction affil
                # chain it feeds is the longer tail), then img half.
                class_ops = [(512, op) for op in range(NOP)] + [
                    (0, op) for op in range(NOP)
                ]

                def emit_class(n):
                    while n > 0 and class_ops:
                        co, op = class_ops.pop(0)
                        src = nat8_sb[:, 0] if co == 0 else nat8_sb[:, 1]
                        nc.tensor.matmul(
                            pcls[0:64, co : co + 512],
                            oh8_sb[:, op, :, :],
                            src[:, op, :, :],
                            start=(op == 0),
                            stop=(op == NOP - 1),
                            perf_mode=DR,
                        )
                        n -= 1

                mFb = statp.tile([64, 1024], f32, tag="mFb")
                mT8 = statp.tile([128, 2, 2, 2, NCLS], fp8, tag="mT8")

                def emit_means(di):
                    # means for direction di (0=img,1=txt) + transpose +
                    # fp8 cast into mT8[:, di]; pm reuses the retired
                    # class psum half.
                    co = 512 * di
                    nc.vector.scalar_tensor_tensor(
                        out=mFb[:, co : co + 512],
                        in0=pcls[0:64, co : co + 512],
                        scalar=mean_imm,
                        in1=aux_sb[:, 0:1].broadcast_to([64, 512]),
                        op0=ALU.mult,
                        op1=ALU.mult,
                    )
                    pm = pcls[:, co : co + 256]
                    for c4 in range(4):
                        nc.tensor.transpose(
                            pm[:, 64 * c4 : 64 * (c4 + 1)],
                            mFb[:, co + 128 * c4 : co + 128 * (c4 + 1)],
                            ident[0:64, 0:64],
                        )
                    nc.vector.tensor_copy(
                        mT8[:, di].rearrange("p b c d -> p (b c d)"), pm[:]
                    )

                # PE warmup during the DMA wait: junk matmuls ramp the
                # pstate so the first real matmuls run at full clock.
                # ones1-based: no dependency on gpsimd's identity build.
                for w in range(30):
                    nc.tensor.matmul(
                        pcls[0:1, 512:513],
                        ones1[:, 0:1],
                        ones1[:, 0:1],
                        start=True,
                        stop=True,
                    )

                with tc.tile_pool(name="psumB", bufs=3, space="PSUM") as psumB:
                    for g in range(NG):
                        colacc = colaccp.tile(
                            [128, GCH], bf16, tag="colacc", name="colacc"
                        )
                        for t in range(RT):
                            ps = psumB.tile([128, GCH], f32, tag="mm", name="ps")
                            for j in range(2):
                                for kp in range(2):
                                    nc.tensor.matmul(
                                        ps[:, 512 * j : 512 * (j + 1)],
                                        is8_sb[:, kp, :, 128 * t : 128 * (t + 1)],
                                        tt8_sb[:, 2 * g + j, kp, :, :],
                                        start=(kp == 0),
                                        stop=(kp == 1),
                                        perf_mode=DR,
                                    )
                                if g == 0 and t == 0 and j == 0:
                                    # G = max(256-col sample) + 6; cross-
                                    # partition max via PE transpose +
                                    # ones-broadcast (no gpsimd library).
                                    nc.vector.tensor_reduce(
                                        Gt[:, 0:1], ps[:, 0:256], axis=X, op=ALU.max
                                    )
                                    nc.vector.tensor_scalar_add(
                                        Gt[:, 0:1], Gt[:, 0:1], 6.0
                                    )
                            if g == 0 and t == 0:
                                nc.tensor.transpose(
                                    pcls[0:1, 896:1024], Gt[:, 0:1], ident[:, :]
                                )
                                nc.vector.tensor_reduce(
                                    gmx[:, :], pcls[0:1, 896:1024], axis=X, op=ALU.max
                                )
                                nc.tensor.matmul(
                                    pcls[:, 895:896],
                                    onesr[:, :],
                                    gmx[:, :],
                                    start=True,
                                    stop=True,
                                )
                                nc.vector.tensor_copy(Gt[:, 0:1], pcls[:, 895:896])
                                nc.vector.tensor_scalar_mul(
                                    Gt[:, 1:2], Gt[:, 0:1], -1.0
                                )
                                nc.vector.tensor_copy(stage[:, 4:5], Gt[:, 0:1])
                            if g == 1 and t in (2, 3):
                                # diag ones-matmuls; must finish before the
                                # class img half resets pcls[:, 0:512]
                                for k in (0, 1) if t == 2 else (2, 3):
                                    nc.tensor.matmul(
                                        pcls[0:1, 0:512],
                                        ones1[:, 0:1],
                                        d4[:, k // 2, k % 2, :],
                                        start=(k == 0),
                                        stop=(k == 3),
                                    )
                                if t == 3:
                                    nc.vector.tensor_copy(
                                        pdg_sb[:], pcls[0:1, 0:512]
                                    )
                                    nc.gpsimd.dma_start(outp[:, :], pdg_sb[:])
                            jk = junkp.tile([128, GCH], bf16, tag="jexp", name="jexp")
                            # rowsums: t==1 via ACT accumulator, others via
                            # DVE accumulators on the colacc op (running
                            # sums; host differences along t).
                            if t == 1:
                                nc.scalar.activation(
                                    jk[:],
                                    ps[:],
                                    Exp,
                                    bias=Gt[:, 1:2],
                                    accum_out=zbG[:, t, g : g + 1],
                                )
                                nc.vector.tensor_tensor(
                                    colacc[:], colacc[:], jk[:], op=ALU.add
                                )
                            elif t == 0:
                                nc.scalar.activation(jk[:], ps[:], Exp, bias=Gt[:, 1:2])
                                nc.vector.tensor_scalar(
                                    colacc[:],
                                    jk[:],
                                    1.0,
                                    0.0,
                                    op0=ALU.mult,
                                    op1=ALU.add,
                                    accum_out=zbG[:, t, g : g + 1],
                                )
                            else:
                                nc.scalar.activation(jk[:], ps[:], Exp, bias=Gt[:, 1:2])
                                nc.vector.scalar_tensor_tensor(
                                    out=colacc[:],
                                    in0=jk[:],
                                    scalar=1.0,
                                    in1=colacc[:],
                                    op0=ALU.mult,
                                    op1=ALU.add,
                                    accum_out=zbG[:, t, g : g + 1],
                                )
                            if g == 0:
                                # diag products, chunked across g0's DVE slack
                                nc.vector.tensor_tensor(
                                    d4[:, t // 2, t % 2, :],
                                    is8_sb[:, t // 2, t % 2, :],
                                    tt8_sb[:, 0, t // 2, t % 2, :],
                                    op=ALU.mult,
                                )
                        nc.gpsimd.dma_start(outc[:, g], colacc[:])
                    emit_class(16)
                    nc.gpsimd.dma_start(outz[:], zbG[:])

                # ---------- affil, per-direction pipeline ----------
                # s-direction first (longer tail: exp + o3b + out3/out4);
                # the img class half runs on PE while the s-direction's
                # ACT/DVE work drains.
                with tc.tile_pool(name="psA", bufs=1, space="PSUM") as psA:
                    pAff = psA.tile([64, 2 * SHARD], f32, tag="pAff", name="pAff")
                    o3s = statp.tile([64, 512], bf16, tag="o3s")
                    o3t = statp.tile([64, 512], bf16, tag="o3t")
                    Esc = junkp.tile([64, SHARD], bf16, tag="Esc", name="Esc")
                    jEt = junkp.tile([64, SHARD], bf16, tag="jEt", name="jEt")

                    emit_means(1)
                    for kp in range(2):
                        nc.tensor.matmul(
                            pAff[:, 0:512],
                            mT8[:, 1, kp],
                            is8_sb[:, kp],
                            start=(kp == 0),
                            stop=(kp == 1),
                            perf_mode=DR,
                        )
                    nc.scalar.activation(
                        Esc[:, :], pAff[:, 0:512], Exp, scale=1.0 / BOOST
                    )
                    nc.vector.tensor_scalar_mul(o3s[:], pAff[:, 0:512], 1.0 / BOOST)
                    # host computes zs = sum_c cnt_c * exp(s)[c, i]
                    nc.gpsimd.dma_start(out4[:], Esc[:, :])
                    nc.gpsimd.dma_start(out3[:, 0:512], o3s[:])

                    # t-direction: img means
                    emit_class(len(class_ops))
                    emit_means(0)
                    for kp in range(2):
                        nc.tensor.matmul(
                            pAff[:, 512:1024],
                            mT8[:, 0, kp],
                            tt8_sb[:, 0, kp],
                            start=(kp == 0),
                            stop=(kp == 1),
                            perf_mode=DR,
                        )
                    nc.scalar.activation(
                        jEt[:, :],
                        pAff[:, 512:1024],
                        Exp,
                        scale=1.0 / BOOST,
                        accum_out=stage[0:64, 5:6],
                    )
                    nc.vector.tensor_scalar_mul(o3t[:], pAff[:, 512:1024], 1.0 / BOOST)
                    nc.gpsimd.dma_start(out3[:, 512:1024], o3t[:])

            nc.gpsimd.dma_start(out[:], stage[:])

    nc.compile()
    return nc


def _combine(outs, outzs, outcs, out3s, out4s, outps, cnt, label):
    o = np.stack([np.asarray(x, dtype=np.float64) for x in outs])  # [8,128,8]
    oz = np.stack([np.asarray(x, dtype=np.float64) for x in outzs])  # [8,128,4,4]
    oc = np.stack([np.asarray(x, dtype=np.float64) for x in outcs])  # [8,128,4,1024]
    o3 = np.stack([np.asarray(x, dtype=np.float64) for x in out3s])  # [8,64,1024]
    o4 = np.stack([np.asarray(x, dtype=np.float64) for x in out4s])  # [8,64,512]
    op = np.stack([np.asarray(x, dtype=np.float64)[0] for x in outps])  # [8,512]

    G = o[:, 0, 4]  # [8]
    diag = op.reshape(B)  # core-major, i-order
    zs = (cnt.astype(np.float64)[None, :, None] * o4).sum(axis=1).reshape(B)
    # column-sum partials: collapse partitions, unrotate windows
    cs_rot = oc.sum(axis=1).reshape(N_CORES, B)  # [8, 4096] rotated cols
    cs = np.empty((N_CORES, B))
    for c in range(N_CORES):
        for w in range(NW):
            r = (w - c) % NW
            cs[c, 512 * w : 512 * (w + 1)] = cs_rot[c, 512 * r : 512 * (r + 1)]
    # row sums: slot t=1 is a direct ACT sum; slots 0/2/3 are DVE running
    # sums of the colacc (R0=A0, R1=Z1, R2=A2-A0-Z1, R3=A3-A2)
    R = np.empty_like(oz)  # [8, 128, t, g]
    R[:, :, 0, :] = oz[:, :, 0, :]
    R[:, :, 1, :] = oz[:, :, 1, :]
    R[:, :, 2, :] = oz[:, :, 2, :] - oz[:, :, 0, :] - oz[:, :, 1, :]
    R[:, :, 3, :] = oz[:, :, 3, :] - oz[:, :, 2, :]
    rowsums = np.empty(B)
    for c in range(N_CORES):
        for t in range(RT):
            rows = slice(SHARD * c + 128 * t, SHARD * c + 128 * (t + 1))
            rowsums[rows] = R[c, :, t, :].sum(axis=1)
    # bf16 exp underflows below ~e^-88; a fully-underflowed row's true
    # LSE sits at the clamp point, so clamping costs <~1 logit there.
    rowsums = np.maximum(rowsums, 1e-38)
    lse1 = np.repeat(G, SHARD) + np.log(rowsums)
    Mg = G.max()
    colsums = np.maximum((cs * np.exp(G - Mg)[:, None]).sum(axis=0), 1e-38)
    lse2 = Mg + np.log(colsums)  # [B]
    loss_i2t = -np.mean(diag - lse1)
    loss_t2i = -np.mean(diag - lse2)
    contr = 0.5 * (loss_i2t + loss_t2i)

    lab = np.asarray(label, dtype=np.int64)
    sT = o3[:, :, 0:512]  # [8, 64, 512]
    tT = o3[:, :, 512:1024]
    tsums = o[:, 0:64, 5]  # [8, 64]
    idx_core = np.arange(B) // SHARD
    idx_i = np.arange(B) % SHARD
    sdiag = sT[idx_core, lab, idx_i]
    tvals = tT[idx_core, lab, idx_i]
    alse = np.log(zs)
    a_i2t = -np.mean(sdiag - alse)
    collse = np.log(tsums.sum(axis=0))  # [64]
    a_t2i = -np.mean(tvals - collse[lab])
    affil = 0.5 * (a_i2t + a_t2i)
    return np.float32(contr + affil)


def kernel(image_feat, text_feat, label, temp, temp2):
    global LAST_RESULTS
    img = np.ascontiguousarray(np.asarray(image_feat, dtype=np.float32))
    txt = np.ascontiguousarray(np.asarray(text_feat, dtype=np.float32))
    labv = np.asarray(label).astype(np.int64).reshape(B)
    tv = float(np.asarray(temp))
    t2v = float(np.asarray(temp2))

    nc = _compiled(tv, t2v)

    import ml_dtypes

    f8 = ml_dtypes.float8_e4m3
    rt_st = float(np.sqrt(1.0 / tv))
    # full text transposed fp8, window-major: [p, w, kp, ki, j]
    ttw = np.ascontiguousarray(
        (txt.T * rt_st)
        .astype(f8)
        .reshape(2, 2, 128, NW, 512)
        .transpose(2, 3, 0, 1, 4)
    )  # [128, 8, 2, 2, 512]

    def _pmT8(x):
        # [512, D] shard -> [p, kp, ki, i] fp8 transposed layout
        return np.ascontiguousarray(
            x.T.reshape(2, 2, 128, SHARD).transpose(2, 0, 1, 3)
        )

    def _nat8(x):
        # [B, D] full -> [p, op, oi, d] fp8 natural layout
        return np.ascontiguousarray(
            x.reshape(NOP, 2, 128, D).transpose(2, 0, 1, 3)
        )

    nat8_in = np.ascontiguousarray(
        np.stack([_nat8(img.astype(f8)), _nat8(txt.astype(f8))], axis=1)
    )
    # one-hot [j, c] in the same j-partition layout as nat8
    ohf = (labv[:, None] == np.arange(NCLS)[None, :]).astype(f8)  # [B, 64]
    oh8_in = np.ascontiguousarray(
        ohf.reshape(NOP, 2, 128, NCLS).transpose(2, 0, 1, 3)
    )
    cnt = np.bincount(labv, minlength=NCLS).astype(np.float32)  # [64]
    aux_in = np.ascontiguousarray(
        np.stack([1.0 / np.maximum(cnt, 1.0), cnt], axis=1)
    )  # [64, 2]

    in_maps = []
    for c in range(N_CORES):
        sl = slice(SHARD * c, SHARD * (c + 1))
        in_maps.append(
            {
                "aux": aux_in,
                "is8": _pmT8((img[sl] * rt_st).astype(f8)),
                "tt8": np.ascontiguousarray(np.roll(ttw, -c, axis=1)),
                "oh8": oh8_in,
                "nat8": nat8_in,
            }
        )

    from concourse import bass_utils

    res = bass_utils.run_bass_kernel_spmd(
        nc, in_maps, core_ids=list(range(N_CORES))
    )
    LAST_RESULTS = res
    return _combine(
        [r["out"] for r in res.results],
        [r["outz"] for r in res.results],
        [r["outc"] for r in res.results],
        [r["out3"] for r in res.results],
        [r["out4"] for r in res.results],
        [r["outp"] for r in res.results],
        cnt,
        labv,
    )
